# revision 17
# baseline (speedup 1.0000x reference)
"""PointPillarsScatter on 8 Trainium2 NeuronCores.

Reference semantics: given voxel_features [P=120000, C=64] and coords
[P, 4] (b, z, y, x) with unique (b, y, x), produce a canvas
(B=4, C=64, NY=512, NX=512) with canvas[b, :, y, x] = voxel_features[p]
and zeros elsewhere.

Strategy (matmul-scatter, fully local per core):
  - Shard the canvas over 8 cores by (batch, y-half): core k owns
    b = k//2, y in [256*(k%2), 256*(k%2)+256).  131072 cells per core.
  - Split the per-core cell range (padded to 131328) into two halves
    (A = cells [0, 65664), B = cells [65664, 131328)).  Pair p covers
    window X = A-cells [384p, 384p+384) and window Y = the same window
    in half B.  Each pair gets K=128 pillar slots shared by both
    windows.
  - Host packs features block-diagonally per pair into a
    partition-major array feat2T [128 slots, NPAIRS, 128]: slot row q,
    cols 0:64 = features if the pillar lies in window X (else 0), cols
    64:128 = features if in window Y.  idxs[q, p] = cell index local
    to its own 384-cell window (float), -1 for empty slots.
  - Kernel per pair: sel[q, j] = (idxs[q, p] == iota[j]) (0/1, exact
    in f32), then matmul: psum = feat_tile^T @ sel gives
    psum[0:64, j]  = channel-major cells of window X and
    psum[64:128, j] = window Y, zeros included.  ACT copies PSUM ->
    SBUF, one DMA per 19-pair group stores [128, 7296] to the stacked
    output [128, 65664] (row q<64: channel q half A; row 64+q:
    channel q half B) with fully contiguous per-partition runs.
  - Host re-stitches the stacked halves and trims to (B, C, NY, NX).

No scratch canvas, no device transposes, no indirect DMA; f32 exact.
"""

import numpy as np

from contextlib import ExitStack

import concourse.bass as bass
import concourse.tile as tile
from concourse import bacc, mybir
from concourse.bass_utils import run_bass_kernel_spmd

# ---- problem constants (hardcoded; harness calls kernel(**inputs)) ----
P_TOT = 120000
C = 64
B = 4
NY = 512
NX = 512

NCORES = 8
YH = NY // 2                  # 256 rows of y per core
NCELLS = YH * NX              # 131072 cells per core
F = 384                       # cells per pair window
NPAIRS = 171                  # windows per half; NPAIRS*F = 65664
HALFW = NPAIRS * F            # 65664 padded cells per half
K = 128                       # pillar slots per pair

GROUP = 19                    # pairs per load/store group (9 groups)
CPB = 4                       # pairs per sel/copy batch

FP32 = mybir.dt.float32

_cached = {}


def _build_program():
    """Build + schedule the SPMD Bass program once."""
    if "nc" in _cached:
        return _cached["nc"]

    nc = bacc.Bacc(
        "TRN2",
        target_bir_lowering=False,
        debug=False,
        enable_asserts=False,
        num_devices=NCORES,
    )

    feat2 = nc.dram_tensor("feat2", [128, NPAIRS, 128], FP32, kind="ExternalInput").ap()
    idxs = nc.dram_tensor("idxs", [128, NPAIRS], FP32, kind="ExternalInput").ap()
    iota = nc.dram_tensor("iota", [128, F], FP32, kind="ExternalInput").ap()
    out = nc.dram_tensor("out", [128, HALFW], FP32, kind="ExternalOutput").ap()

    with tile.TileContext(nc) as tc, ExitStack() as ctx:
        constp = ctx.enter_context(tc.tile_pool(name="const", bufs=1))
        featp = ctx.enter_context(tc.tile_pool(name="featp", bufs=3))
        selp = ctx.enter_context(tc.tile_pool(name="selp", bufs=4))
        psump = ctx.enter_context(tc.tile_pool(name="psump", bufs=2, space="PSUM"))
        outp = ctx.enter_context(tc.tile_pool(name="outp", bufs=2))

        # loads issue from the ACT HWDGE ring, stores from the SP ring —
        # sharing one ring head-of-line-blocks prefetches behind stores.
        iota_sb = constp.tile([128, F], FP32)
        nc.scalar.dma_start(out=iota_sb[:], in_=iota[:])
        idx_sb = constp.tile([128, NPAIRS], FP32)
        nc.scalar.dma_start(out=idx_sb[:], in_=idxs[:])

        gsizes = [GROUP] * 9
        assert sum(gsizes) == NPAIRS

        pair0 = 0
        for gsize in gsizes:
            # ---- load this group's packed features (contiguous per partition)
            feat_full = featp.tile([128, GROUP * 128], FP32, tag="feat")
            feat_sb = feat_full[:, : gsize * 128]
            nc.scalar.dma_start(
                out=feat_sb.rearrange("q (g c) -> q g c", g=gsize),
                in_=feat2[:, pair0 : pair0 + gsize, :],
            )
            ostage_full = outp.tile([128, GROUP * F], FP32, tag="ostage")
            ostage = ostage_full[:, : gsize * F]

            lp = 0
            while lp < gsize:
                nb = min(CPB, gsize - lp)
                p = pair0 + lp
                # ---- selection matrices for nb pairs: [128, nb, F] (DVE;
                # Pool's is_equal ucode measured 16 cyc/elem — unusable)
                sel = selp.tile([128, CPB * F], FP32)
                sel_v = sel[:, : nb * F].rearrange("q (n f) -> q n f", n=nb)
                in0 = idx_sb[:, p : p + nb].unsqueeze(2).to_broadcast(
                    [128, nb, F]
                )
                in1 = iota_sb[:].unsqueeze(1).to_broadcast([128, nb, F])
                nc.vector.tensor_tensor(
                    out=sel_v, in0=in0, in1=in1, op=mybir.AluOpType.is_equal
                )
                # ---- one matmul per pair into a 4-bank PSUM tile
                ps = psump.tile([128, CPB * 512], FP32)
                for j in range(nb):
                    nc.tensor.matmul(
                        ps[:, j * 512 : j * 512 + F],
                        feat_sb[:, (lp + j) * 128 : (lp + j + 1) * 128],
                        sel[:, j * F : (j + 1) * F],
                        start=True,
                        stop=True,
                    )
                # ---- PSUM -> SBUF staging copy on ACT (batched over nb pairs)
                cp_out = ostage[:, lp * F : (lp + nb) * F].rearrange(
                    "q (n f) -> q n f", n=nb
                )
                cp_in = ps[:].rearrange("q (n x) -> q n x", x=512)[:, :nb, :F]
                nc.scalar.copy(out=cp_out, in_=cp_in)
                lp += nb

            # ---- store in 2 chunks for finer overlap
            csz = max(1, (gsize + 1) // 2)
            st = 0
            while st < gsize:
                en = min(st + csz, gsize)
                nc.sync.dma_start(
                    out=out[:, (pair0 + st) * F : (pair0 + en) * F],
                    in_=ostage[:, st * F : en * F],
                )
                st = en
            pair0 += gsize

    nc.compile()
    _cached["nc"] = nc
    return nc


def _prep_core_inputs(vf, cells, core_mask):
    """Pack one core's pillars into feat2T/idxs arrays."""
    feat2 = np.zeros((128, NPAIRS, 128), dtype=np.float32)
    idxs = np.full((128, NPAIRS), -1.0, dtype=np.float32)

    cells_k = cells[core_mask]
    feats_k = vf[core_mask]

    half = (cells_k >= HALFW).astype(np.int64)      # 0 = window X, 1 = Y
    local_h = cells_k - half * HALFW
    pair_id = local_h // F
    local = (local_h - pair_id * F).astype(np.float32)

    order = np.argsort(pair_id, kind="stable")
    cells_k = cells_k[order]
    feats_k = feats_k[order]
    half = half[order]
    pair_id = pair_id[order]
    local = local[order]

    counts = np.bincount(pair_id, minlength=NPAIRS)
    assert counts.max() <= K, f"pair overflow: {counts.max()} > {K}"
    starts = np.zeros(NPAIRS, dtype=np.int64)
    starts[1:] = np.cumsum(counts)[:-1]
    slot = np.arange(len(cells_k)) - starts[pair_id]

    col_base = half * 64
    for cb in (0, 64):
        m = col_base == cb
        feat2[slot[m], pair_id[m], cb : cb + 64] = feats_k[m]
    idxs[slot, pair_id] = local
    return feat2, idxs


def kernel(voxel_features, coords, batch_size, nx, ny):
    vf = np.ascontiguousarray(np.asarray(voxel_features), dtype=np.float32)
    co = np.asarray(coords).astype(np.int32)
    assert int(batch_size) == B and int(nx) == NX and int(ny) == NY
    assert vf.shape == (P_TOT, C)

    b = co[:, 0]
    y = co[:, 2]
    x = co[:, 3]
    core = b * 2 + (y >= YH).astype(np.int32)
    cells = (y % YH).astype(np.int64) * NX + x

    iota = np.broadcast_to(
        np.arange(F, dtype=np.float32)[None, :], (128, F)
    ).copy()

    in_maps = []
    for k in range(NCORES):
        feat2, idxs = _prep_core_inputs(vf, cells, core == k)
        in_maps.append({"feat2": feat2, "idxs": idxs, "iota": iota})

    nc = _build_program()
    res = run_bass_kernel_spmd(nc, in_maps, list(range(NCORES))).results

    out_full = np.empty((B, C, NY, NX), dtype=np.float32)
    for k in range(NCORES):
        bb, h = k // 2, k % 2
        stacked = res[k]["out"]                      # [128, 65664]
        flat = stacked.reshape(2, C, HALFW).transpose(1, 0, 2).reshape(C, 2 * HALFW)
        out_full[bb, :, h * YH : (h + 1) * YH, :] = flat[:, :NCELLS].reshape(
            C, YH, NX
        )
    return out_full


# revision 18
# speedup vs baseline: 1.1032x; 1.1032x over previous
"""PointPillarsScatter on 8 Trainium2 NeuronCores.

Reference semantics: given voxel_features [P=120000, C=64] and coords
[P, 4] (b, z, y, x) with unique (b, y, x), produce a canvas
(B=4, C=64, NY=512, NX=512) with canvas[b, :, y, x] = voxel_features[p]
and zeros elsewhere.

Strategy (matmul-scatter, fully local per core):
  - Shard the canvas over 8 cores by (batch, y-half): core k owns
    b = k//2, y in [256*(k%2), 256*(k%2)+256).  131072 cells per core.
  - Split the per-core cell range (padded to 131328) into two halves
    (A = cells [0, 65664), B = cells [65664, 131328)).  Pair p covers
    window X = A-cells [384p, 384p+384) and window Y = the same window
    in half B.  Each pair gets K=128 pillar slots shared by both
    windows.
  - Host packs features block-diagonally per pair into a
    partition-major array feat2T [128 slots, NPAIRS, 128]: slot row q,
    cols 0:64 = features if the pillar lies in window X (else 0), cols
    64:128 = features if in window Y.  idxs[q, p] = cell index local
    to its own 384-cell window (float), -1 for empty slots.
  - Kernel per pair: sel[q, j] = (idxs[q, p] == iota[j]) (0/1, exact
    in f32), then matmul: psum = feat_tile^T @ sel gives
    psum[0:64, j]  = channel-major cells of window X and
    psum[64:128, j] = window Y, zeros included.  ACT copies PSUM ->
    SBUF, one DMA per 19-pair group stores [128, 7296] to the stacked
    output [128, 65664] (row q<64: channel q half A; row 64+q:
    channel q half B) with fully contiguous per-partition runs.
  - Host re-stitches the stacked halves and trims to (B, C, NY, NX).

No scratch canvas, no device transposes, no indirect DMA; f32 exact.
"""

import numpy as np

from contextlib import ExitStack

import concourse.bass as bass
import concourse.tile as tile
from concourse import bacc, mybir
from concourse.bass_utils import run_bass_kernel_spmd

# ---- problem constants (hardcoded; harness calls kernel(**inputs)) ----
P_TOT = 120000
C = 64
B = 4
NY = 512
NX = 512

NCORES = 8
YH = NY // 2                  # 256 rows of y per core
NCELLS = YH * NX              # 131072 cells per core
F = 384                       # cells per pair window
NPAIRS = 171                  # windows per half; NPAIRS*F = 65664
HALFW = NPAIRS * F            # 65664 padded cells per half
K = 128                       # pillar slots per pair

GROUP = 19                    # pairs per load/store group (9 groups)
CPB = 4                       # pairs per sel/copy batch

FP32 = mybir.dt.float32

_cached = {}


def _build_program():
    """Build + schedule the SPMD Bass program once."""
    if "nc" in _cached:
        return _cached["nc"]

    nc = bacc.Bacc(
        "TRN2",
        target_bir_lowering=False,
        debug=False,
        enable_asserts=False,
        num_devices=NCORES,
    )

    feat2 = nc.dram_tensor("feat2", [128, NPAIRS, 128], FP32, kind="ExternalInput").ap()
    idxs = nc.dram_tensor("idxs", [128, NPAIRS], FP32, kind="ExternalInput").ap()
    iota = nc.dram_tensor("iota", [128, F], FP32, kind="ExternalInput").ap()
    out = nc.dram_tensor("out", [128, HALFW], FP32, kind="ExternalOutput").ap()

    with tile.TileContext(nc) as tc, ExitStack() as ctx:
        constp = ctx.enter_context(tc.tile_pool(name="const", bufs=1))
        featp = ctx.enter_context(tc.tile_pool(name="featp", bufs=3))
        selp = ctx.enter_context(tc.tile_pool(name="selp", bufs=4))
        psump = ctx.enter_context(tc.tile_pool(name="psump", bufs=2, space="PSUM"))
        outp = ctx.enter_context(tc.tile_pool(name="outp", bufs=2))

        # loads issue from the ACT HWDGE ring, stores from the SP ring —
        # sharing one ring head-of-line-blocks prefetches behind stores.
        iota_sb = constp.tile([128, F], FP32)
        nc.scalar.dma_start(out=iota_sb[:], in_=iota[:])
        idx_sb = constp.tile([128, NPAIRS], FP32)
        nc.scalar.dma_start(out=idx_sb[:], in_=idxs[:])

        gsizes = [GROUP] * 9
        assert sum(gsizes) == NPAIRS

        pair0 = 0
        for gsize in gsizes:
            # ---- load this group's packed features (contiguous per partition)
            feat_full = featp.tile([128, GROUP * 128], FP32, tag="feat")
            feat_sb = feat_full[:, : gsize * 128]
            nc.scalar.dma_start(
                out=feat_sb.rearrange("q (g c) -> q g c", g=gsize),
                in_=feat2[:, pair0 : pair0 + gsize, :],
            )
            ostage_full = outp.tile([128, GROUP * F], FP32, tag="ostage")
            ostage = ostage_full[:, : gsize * F]

            lp = 0
            while lp < gsize:
                nb = min(CPB, gsize - lp)
                p = pair0 + lp
                # ---- selection matrices for nb pairs: [128, nb, F] (DVE;
                # Pool's is_equal ucode measured 16 cyc/elem — unusable)
                sel = selp.tile([128, CPB * F], FP32)
                sel_v = sel[:, : nb * F].rearrange("q (n f) -> q n f", n=nb)
                in0 = idx_sb[:, p : p + nb].unsqueeze(2).to_broadcast(
                    [128, nb, F]
                )
                in1 = iota_sb[:].unsqueeze(1).to_broadcast([128, nb, F])
                nc.vector.tensor_tensor(
                    out=sel_v, in0=in0, in1=in1, op=mybir.AluOpType.is_equal
                )
                # ---- one matmul per pair into a 4-bank PSUM tile
                ps = psump.tile([128, CPB * 512], FP32)
                for j in range(nb):
                    nc.tensor.matmul(
                        ps[:, j * 512 : j * 512 + F],
                        feat_sb[:, (lp + j) * 128 : (lp + j + 1) * 128],
                        sel[:, j * F : (j + 1) * F],
                        start=True,
                        stop=True,
                    )
                # ---- PSUM -> SBUF staging copy on ACT (batched over nb pairs)
                cp_out = ostage[:, lp * F : (lp + nb) * F].rearrange(
                    "q (n f) -> q n f", n=nb
                )
                cp_in = ps[:].rearrange("q (n x) -> q n x", x=512)[:, :nb, :F]
                nc.scalar.copy(out=cp_out, in_=cp_in)
                # ---- store this batch right away (trails the copy by one)
                nc.sync.dma_start(
                    out=out[:, p * F : (p + nb) * F],
                    in_=ostage[:, lp * F : (lp + nb) * F],
                )
                lp += nb

            pair0 += gsize

    nc.compile()
    _cached["nc"] = nc
    return nc


def _prep_core_inputs(vf, cells, core_mask):
    """Pack one core's pillars into feat2T/idxs arrays."""
    feat2 = np.zeros((128, NPAIRS, 128), dtype=np.float32)
    idxs = np.full((128, NPAIRS), -1.0, dtype=np.float32)

    cells_k = cells[core_mask]
    feats_k = vf[core_mask]

    half = (cells_k >= HALFW).astype(np.int64)      # 0 = window X, 1 = Y
    local_h = cells_k - half * HALFW
    pair_id = local_h // F
    local = (local_h - pair_id * F).astype(np.float32)

    order = np.argsort(pair_id, kind="stable")
    cells_k = cells_k[order]
    feats_k = feats_k[order]
    half = half[order]
    pair_id = pair_id[order]
    local = local[order]

    counts = np.bincount(pair_id, minlength=NPAIRS)
    assert counts.max() <= K, f"pair overflow: {counts.max()} > {K}"
    starts = np.zeros(NPAIRS, dtype=np.int64)
    starts[1:] = np.cumsum(counts)[:-1]
    slot = np.arange(len(cells_k)) - starts[pair_id]

    col_base = half * 64
    for cb in (0, 64):
        m = col_base == cb
        feat2[slot[m], pair_id[m], cb : cb + 64] = feats_k[m]
    idxs[slot, pair_id] = local
    return feat2, idxs


def kernel(voxel_features, coords, batch_size, nx, ny):
    vf = np.ascontiguousarray(np.asarray(voxel_features), dtype=np.float32)
    co = np.asarray(coords).astype(np.int32)
    assert int(batch_size) == B and int(nx) == NX and int(ny) == NY
    assert vf.shape == (P_TOT, C)

    b = co[:, 0]
    y = co[:, 2]
    x = co[:, 3]
    core = b * 2 + (y >= YH).astype(np.int32)
    cells = (y % YH).astype(np.int64) * NX + x

    iota = np.broadcast_to(
        np.arange(F, dtype=np.float32)[None, :], (128, F)
    ).copy()

    in_maps = []
    for k in range(NCORES):
        feat2, idxs = _prep_core_inputs(vf, cells, core == k)
        in_maps.append({"feat2": feat2, "idxs": idxs, "iota": iota})

    nc = _build_program()
    res = run_bass_kernel_spmd(nc, in_maps, list(range(NCORES))).results

    out_full = np.empty((B, C, NY, NX), dtype=np.float32)
    for k in range(NCORES):
        bb, h = k // 2, k % 2
        stacked = res[k]["out"]                      # [128, 65664]
        flat = stacked.reshape(2, C, HALFW).transpose(1, 0, 2).reshape(C, 2 * HALFW)
        out_full[bb, :, h * YH : (h + 1) * YH, :] = flat[:, :NCELLS].reshape(
            C, YH, NX
        )
    return out_full
